# revision 14
# baseline (speedup 1.0000x reference)
# Trainium2 Bass kernel for nn_BasicTransformerBlockST (spatio-temporal
# transformer block: windowed spatial self-attention, two temporal
# self-attentions with relative-position bias + causal mask, cross-attention
# to a text context, and a GEGLU feed-forward).
#
# Sharding: data-parallel over the 128 (b, nh, nw) spatial windows -> 16
# windows x 4096 tokens per core; every stage (window attn / temporal attn /
# cross attn / FF) is closed under this shard, so no collectives are needed.
#
# v2: all five stages are fused into one per-chunk pipeline (weights all
# resident) so the Tile scheduler can overlap chunk c+1's matmuls with chunk
# c's vector/scalar phases and the PE never idles long enough to re-throttle
# (HAM). The residual stream xT lives in SBUF as bf16 [128, 3, ntok]; LN
# stats are computed at partition 64 directly in PSUM (no partition-0->64
# shuffles); LN beta rides the psum evacuation as a per-partition bias for
# q/k, is folded into bo on the host for v, and keeps the augmented-row path
# only for the FF stage. rstd = exp(-.5 ln(var+eps)) keeps ACT on the
# ln/exp table set shared with the attention exps.
import numpy as np
import ml_dtypes

import concourse.bass as bass
import concourse.tile as tile
from concourse import bacc, mybir
from concourse.bass_utils import run_bass_kernel_spmd

F32 = mybir.dt.float32
BF16 = mybir.dt.bfloat16
AF = mybir.ActivationFunctionType
ALU = mybir.AluOpType

# Make Ln and Exp resolve to the one ACT table set that contains both
# (natural_log_exp_and_others); otherwise the table-load placement pass
# assigns Ln -> natural_log and Exp -> exp_and_others and every LN tile
# pays two ~1.3us ACT table reloads back-to-back.
_orig_gat = bacc.get_activation_tables


def _gat_ln_exp_combined(arch):
    tables = _orig_gat(arch)
    for name, funcs in tables.items():
        if name != 'natural_log_exp_and_others':
            funcs.discard(AF.Exp)
            funcs.discard(AF.Ln)
    return tables


bacc.get_activation_tables = _gat_ln_exp_combined

D, CTX_DIM, HEADS, DH, T_LEN, WS, MAXREL, FF = 320, 768, 8, 40, 16, 4, 16, 1280
B, H, W = 2, 32, 32
NH = H // WS
NWIN = B * NH * NH          # 128 windows total
NCORES = 8
SEQ_TOK = T_LEN * WS * WS   # 256 tokens per window
SCALE = DH ** -0.5
NEG = -1e5
NCTX = 77
EPS = 1e-5

bfdt = ml_dtypes.bfloat16


# ----------------------------------------------------------------------------
# host-side data prep
# ----------------------------------------------------------------------------

def shard_x(x, win_per_core):
    xr = np.asarray(x, np.float32).reshape(B, D, T_LEN, NH, WS, NH, WS)
    xr = xr.transpose(0, 3, 5, 1, 4, 6, 2)          # B nh nw C wh ww T
    xr = xr.reshape(NWIN, D, WS * WS * T_LEN)       # win C (s t)
    ncore = NWIN // win_per_core
    xr = xr.reshape(ncore, win_per_core, D, WS * WS * T_LEN)
    xr = xr.transpose(0, 2, 1, 3).reshape(ncore, D, win_per_core * WS * WS * T_LEN)
    return np.ascontiguousarray(xr)


def unshard_x(shards, win_per_core):
    ncore = NWIN // win_per_core
    xr = np.asarray(shards, np.float32)
    xr = xr.reshape(ncore, D, win_per_core, WS * WS * T_LEN).transpose(0, 2, 1, 3)
    xr = xr.reshape(B, NH, NH, D, WS, WS, T_LEN)
    xr = xr.transpose(0, 3, 6, 1, 4, 2, 5)          # B C T nh wh nw ww
    return np.ascontiguousarray(xr.reshape(B, D, T_LEN, H, W))


def _cmajor(a, rows):
    """[rows_logical<=rows, cols] -> [128, rows/128, cols], zero padded."""
    out = np.zeros((rows, a.shape[1]), np.float32)
    out[: a.shape[0]] = a
    return np.ascontiguousarray(
        out.reshape(rows // 128, 128, a.shape[1]).transpose(1, 0, 2))


VDIM = DH + 1    # 41 per-head value columns; slot 32 is the ones column


def vslot(c):
    """map v-slot index c in [0,41) to head dim, or None for the ones slot."""
    if c == 32:
        return None
    return c if c < 32 else c - 1


def pad_v_cols(Wv):
    """[cin, 320] -> [cin, 328]: per-head 41 columns; slot 32 left zero
    (filled with ones on device for the softmax-denominator trick)."""
    cin = Wv.shape[0]
    out = np.zeros((cin, HEADS * VDIM), np.float32)
    for h in range(HEADS):
        for c in range(VDIM):
            d = vslot(c)
            if d is not None:
                out[:, h * VDIM + c] = Wv[:, h * DH + d]
    return out


def pad_head_cols(Wx):
    """[cin, 320] -> [cin, 512]: head h cols at h*64+[0,40), zeros between."""
    out = np.zeros((Wx.shape[0], 512), np.float32)
    for h in range(HEADS):
        out[:, h * 64: h * 64 + 40] = Wx[:, h * 40: (h + 1) * 40]
    return out


def prep_proj_w(Wraw, gamma, extra_row=None, pad_heads=False):
    """Augmented c-major projection weight [128, 3, dout]:
    rows 0..320 = W*gamma[:,None]; row (2,64) = colsum (pairs with the -mu
    value the device writes into xT row (2,64), scaled by rstd in nhat);
    row (2,65) = extra_row (FF only: beta@Wg + b1, pairs with a ones row)."""
    Wg = np.asarray(Wraw, np.float32) * np.asarray(gamma, np.float32)[:, None]
    if pad_heads:
        Wg = pad_head_cols(Wg)
    out = np.zeros((384, Wg.shape[1]), np.float32)
    out[:320] = Wg
    out[256 + 64] = Wg.sum(0)
    if extra_row is not None:
        out[256 + 65] = extra_row
    return _cmajor(out, 384)


def prep_wo(Wo, bo):
    """[320, 320] -> lhsT [128, 5, 320]: head h rows at h*64+c for v-slot c
    (zero at the sum slot c=32); bo rides in the unused row 63 of ptile 3
    (pairs with a device ones row there) so no extra K=1 matmul is needed."""
    out = np.zeros((640, 320), np.float32)
    Wo = np.asarray(Wo, np.float32)
    for h in range(HEADS):
        for c in range(VDIM):
            d = vslot(c)
            if d is not None:
                out[h * 64 + c] = Wo[h * DH + d]
    out[3 * 128 + 63] = np.asarray(bo, np.float32)
    return _cmajor(out, 640)


def prep_tabq(table):
    """relk [33, 40] -> tabQ [128, 256]: col (i*16+J) holds table[J-i+16] in
    rows 0..40 AND a copy in rows 64..104 (so lhsT base matches q's base)."""
    out = np.zeros((128, 256), np.float32)
    t = np.asarray(table, np.float32)
    for i in range(T_LEN):
        for J in range(T_LEN):
            out[:40, i * 16 + J] = t[J - i + MAXREL]
            out[64:104, i * 16 + J] = t[J - i + MAXREL]
    return out


def prep_tvrep(table):
    """relv [33, 40] -> tvrep [128, 16*41]: for query pos i, col i*41+c
    (v-slot c; zero at c=32) row (s*16+j) holds table[j-i+16, dim(c)]."""
    out = np.zeros((128, T_LEN * VDIM), np.float32)
    t = np.asarray(table, np.float32)
    for i in range(T_LEN):
        for s in range(8):
            for j in range(T_LEN):
                for c in range(VDIM):
                    d = vslot(c)
                    if d is not None:
                        out[s * 16 + j, i * VDIM + c] = t[j - i + MAXREL, d]
    return out


def prep_kaug():
    """constant selector [128, 128]: rows b+J (for each base b in
    0/32/64/96) one at cols (s*16+J)."""
    out = np.zeros((128, 128), np.float32)
    for base in (0, 32, 64, 96):
        for s in range(8):
            for J in range(T_LEN):
                out[base + J, s * 16 + J] = 1.0
    return out


def prep_mask():
    """additive [128, 4, 128]: 4 copies of the block-causal mask (so one
    tensor_tensor covers a 4-group score tile)."""
    m = np.full((128, 128), NEG, np.float32)
    for s in range(8):
        for j in range(T_LEN):
            m[s * 16 + j, s * 16 + j: (s + 1) * 16] = 0.0
    return np.ascontiguousarray(np.tile(m, (1, 4)).reshape(128, 4, 128))


def prep_selw():
    """[8, 4, 128]: selector lhsT mapping the 8 per-head reciprocal rows to
    the chunkbuf partition layout (head (pt, lo) rows 0..41, (pt, hi) 64..105).
    rec8 row pt = head (pt, lo); row 4+pt = head (pt, hi)."""
    out = np.zeros((8, 4, 128), np.float32)
    for pt in range(4):
        out[pt, pt, 0: VDIM] = 1.0
        out[4 + pt, pt, 64: 64 + VDIM] = 1.0
    return out


def head_pad_bias(b320):
    """[320] -> [128, 4] c-major per-partition bias planes for the
    head-padded 512-col q/k layout."""
    b512 = np.zeros(512, np.float32)
    for h in range(HEADS):
        b512[h * 64: h * 64 + 40] = b320[h * 40: (h + 1) * 40]
    return np.ascontiguousarray(b512.reshape(4, 128).T)


def prep_weights(wd):
    t = {}

    def bfc(x):
        return np.ascontiguousarray(np.asarray(x, np.float32).astype(bfdt))

    qkbias = np.zeros((128, 8, 4), np.float32)
    for si, (nm, gk, bk) in enumerate((('a1', 'ln1_g', 'ln1_b'),
                                       ('t1', 'ln4_g', 'ln4_b'),
                                       ('t2', 'ln5_g', 'ln5_b'))):
        g = np.asarray(wd[gk], np.float32)
        beta = np.asarray(wd[bk], np.float32)
        Wq = np.asarray(wd[f'{nm}_Wq'], np.float32)
        Wk = np.asarray(wd[f'{nm}_Wk'], np.float32)
        Wv = np.asarray(wd[f'{nm}_Wv'], np.float32)
        Wo = np.asarray(wd[f'{nm}_Wo'], np.float32)
        t[f'{nm}_wq'] = bfc(prep_proj_w(Wq, g, pad_heads=True))
        t[f'{nm}_wk'] = bfc(prep_proj_w(Wk, g, pad_heads=True))
        t[f'{nm}_wv'] = bfc(prep_proj_w(pad_v_cols(Wv), g))
        # v-side LN beta folds exactly into bo (attention rows sum to 1):
        bo2 = np.asarray(wd[f'{nm}_bo'], np.float32) + (beta @ (Wv * g[:, None])) @ Wo
        t[f'{nm}_wo'] = bfc(prep_wo(Wo, bo2))
        # q/k-side beta applied as per-partition bias at psum evacuation
        qkbias[:, 2 * si, :] = head_pad_bias(beta @ (Wq * g[:, None]))
        qkbias[:, 2 * si + 1, :] = head_pad_bias(beta @ (Wk * g[:, None]))
    g2 = np.asarray(wd['ln2_g'], np.float32)
    b2 = np.asarray(wd['ln2_b'], np.float32)
    t['a2_wq'] = bfc(prep_proj_w(wd['a2_Wq'], g2, pad_heads=True))
    qkbias[:, 6, :] = head_pad_bias(b2 @ (np.asarray(wd['a2_Wq'], np.float32) * g2[:, None]))
    t['a2_wk'] = bfc(_cmajor(pad_head_cols(np.asarray(wd['a2_Wk'], np.float32)), 768))
    t['a2_wv'] = bfc(_cmajor(pad_v_cols(np.asarray(wd['a2_Wv'], np.float32)), 768))
    t['a2_wo'] = bfc(prep_wo(wd['a2_Wo'], wd['a2_bo']))
    t['qkbias'] = np.ascontiguousarray(qkbias)
    # FF keeps the full augmented-row path (beta@W + b1 at row (2,65),
    # paired with a ones row the device DMAs into nhat).
    g3 = np.asarray(wd['ln3_g'], np.float32)
    b3 = np.asarray(wd['ln3_b'], np.float32)
    W1g = np.asarray(wd['ff_W1'], np.float32) * g3[:, None]
    t['ff_w1'] = bfc(prep_proj_w(wd['ff_W1'], g3,
                                 extra_row=b3 @ W1g + np.asarray(wd['ff_b1'], np.float32)))
    W2aug = np.zeros((1408, 320), np.float32)
    W2aug[:1280] = np.asarray(wd['ff_W2'], np.float32)
    W2aug[1280] = np.asarray(wd['ff_b2'], np.float32)
    t['ff_w2'] = bfc(_cmajor(W2aug, 1408))
    t['t1_tabq'] = bfc(prep_tabq(wd['t1_relk']))
    t['t2_tabq'] = bfc(prep_tabq(wd['t2_relk']))
    t['t1_tvrep'] = bfc(prep_tvrep(wd['t1_relv']))
    t['t2_tvrep'] = bfc(prep_tvrep(wd['t2_relv']))
    t['kaug'] = bfc(prep_kaug())
    t['mask'] = prep_mask()
    t['selw'] = bfc(prep_selw())
    return t


WEIGHT_SHAPES = {}
for _s in ('a1', 't1', 't2'):
    WEIGHT_SHAPES.update({f'{_s}_wq': ([128, 3, 512], BF16),
                          f'{_s}_wk': ([128, 3, 512], BF16),
                          f'{_s}_wv': ([128, 3, 328], BF16),
                          f'{_s}_wo': ([128, 5, 320], BF16)})
WEIGHT_SHAPES.update({
    'a2_wq': ([128, 3, 512], BF16), 'a2_wk': ([128, 6, 512], BF16),
    'a2_wv': ([128, 6, 328], BF16), 'a2_wo': ([128, 5, 320], BF16),
    'ff_w1': ([128, 3, 2560], BF16), 'ff_w2': ([128, 11, 320], BF16),
    't1_tabq': ([128, 256], BF16), 't2_tabq': ([128, 256], BF16),
    't1_tvrep': ([128, 656], BF16), 't2_tvrep': ([128, 656], BF16),
    'kaug': ([128, 128], BF16), 'mask': ([128, 4, 128], F32),
    'selw': ([8, 4, 128], BF16), 'qkbias': ([128, 8, 4], F32),
})


# ----------------------------------------------------------------------------
# device kernel builder
# ----------------------------------------------------------------------------

def hrow(h):
    """(ptile, row0) of head h in the head-padded 512-row q/k layout."""
    return h // 2, (h % 2) * 64


def mkap(t, extra_off, dims):
    return bass.AP(tensor=t.tensor, offset=t.offset + extra_off, ap=[list(d) for d in dims])


class Builder:
    def __init__(self, nwin=16, chunk_win=4):
        self.nwin = nwin
        self.ntok = nwin * SEQ_TOK
        self.chunk = min(chunk_win * SEQ_TOK, self.ntok)   # tokens per chunk
        self.KS_AUG = ((0, 128), (1, 128), (2, 65))
        self.KS_FF = ((0, 128), (1, 128), (2, 66))

    def build(self, num_devices=1):
        nc = bacc.Bacc("TRN2", target_bir_lowering=False, debug=False,
                       num_devices=num_devices)
        self.nc = nc
        dram = {}
        dram['xT'] = nc.declare_dram_parameter('xT', [128, 3, self.ntok], BF16,
                                               isOutput=False)
        dram['ctxT'] = nc.declare_dram_parameter('ctxT', [128, 6, NCTX], BF16,
                                                 isOutput=False)
        for nm, (shp, dt) in WEIGHT_SHAPES.items():
            dram[nm] = nc.declare_dram_parameter(nm, list(shp), dt, isOutput=False)
        out_yT = nc.declare_dram_parameter('yT', [128, 3, self.ntok], BF16,
                                           isOutput=True)
        self.dram = dram
        with tile.TileContext(nc) as tc:
            self.tc = tc
            self._emit(out_yT)
        nc.compile()
        return nc

    # ---------------- top level ----------------
    def _emit(self, out_yT):
        from contextlib import ExitStack
        nc, tc = self.nc, self.tc
        with ExitStack() as ctx:
            resid = ctx.enter_context(tc.tile_pool(name="resid", bufs=1))
            consts = ctx.enter_context(tc.tile_pool(name="consts", bufs=1))
            wpool = ctx.enter_context(tc.tile_pool(name="wpool", bufs=1))

            xT = resid.tile([128, 3, self.ntok], BF16)
            nc.sync.dma_start(out=xT, in_=self.dram['xT'][:])
            self.xT = xT

            self.w = {}
            for nm, (shp, dt) in WEIGHT_SHAPES.items():
                self.w[nm] = wpool.tile(list(shp), dt, name=f"sb_{nm}")
                nc.sync.dma_start(out=self.w[nm], in_=self.dram[nm][:])

            self.ones_col_bf = consts.tile([128, 1], BF16, name="onescolbf")
            nc.vector.memset(self.ones_col_bf, 1.0)
            self.eps_col = consts.tile([128, 1], F32, name="epscol")
            nc.vector.memset(self.eps_col, EPS)
            self.ones_row_bf = consts.tile([1, 512], BF16, name="onesrowbf")
            nc.vector.memset(self.ones_row_bf, 1.0)
            self.mask = self.w['mask']
            self.kaug = self.w['kaug']
            self.selw = self.w['selw']
            self.ctxT = self.w['ctxT'] = consts.tile([128, 6, NCTX], BF16,
                                                     name="ctxTc")
            nc.sync.dma_start(out=self.ctxT, in_=self.dram['ctxT'][:])

            pools = {
                'psum': ctx.enter_context(tc.tile_pool(name="ps8", bufs=1, space="PSUM")),
                'nhat': ctx.enter_context(tc.tile_pool(name="nhatp", bufs=3)),
                'ln': ctx.enter_context(tc.tile_pool(name="lnp", bufs=2)),
                'attn': ctx.enter_context(tc.tile_pool(name="attnp", bufs=2)),
                'qkv': ctx.enter_context(tc.tile_pool(name="qkvp", bufs=2)),
                'cb': ctx.enter_context(tc.tile_pool(name="cbp", bufs=2)),
                'ep': ctx.enter_context(tc.tile_pool(name="epp", bufs=4)),
                'ff': ctx.enter_context(tc.tile_pool(name="ffp", bufs=2)),
                'kv2': ctx.enter_context(tc.tile_pool(name="kv2p", bufs=1)),
                'big1': ctx.enter_context(tc.tile_pool(name="big1p", bufs=1)),
            }
            self.pools = pools

            self.prep_cross_kv()
            for c0 in range(0, self.ntok, self.chunk):
                self.stage_window(c0)
                self.stage_temporal(c0, 't1')
                self.stage_cross(c0)
                self.stage_temporal(c0, 't2')
                self.stage_ff(c0)

            nc.sync.dma_start(out=out_yT[:], in_=xT)

    # ---------------- layernorm ----------------
    def ln_nhat(self, tok0, ncols, ff=False):
        """nhat [128, 3, ncols] bf16: rows 0..320 = (x-mu)*rstd; row (2,64)
        = -mu*rstd (via -mu scribbled into xT row (2,64)); FF additionally
        gets a ones row at (2,65) for the beta/b1 augmented contraction."""
        nc = self.nc
        xT = self.xT
        pools = self.pools
        cols = slice(tok0, tok0 + ncols)
        ps_sum = pools['psum'].tile([128, 512], F32, tag="pc")
        ps_sq = pools['psum'].tile([128, 512], F32, tag="pd")
        for k in range(3):
            rows = 128 if k < 2 else 64
            sq = pools['ln'].tile([128, 512], BF16, tag="lnsq")
            nc.gpsimd.tensor_tensor(out=sq[:rows, :ncols],
                                    in0=xT[:rows, k, cols],
                                    in1=xT[:rows, k, cols], op=ALU.mult)
            nc.tensor.matmul(ps_sum[0:1, :ncols],
                             self.ones_col_bf[:rows],
                             xT[:rows, k, cols],
                             start=(k == 0), stop=(k == 2),
                             tile_position=(0, 0))
            nc.tensor.matmul(ps_sq[0:1, :ncols],
                             self.ones_col_bf[:rows],
                             sq[:rows, :ncols],
                             start=(k == 0), stop=(k == 2),
                             tile_position=(0, 0))
        # narrow chain at partition 0 (both stat banks put row 0 there):
        # nmu <- -mu; ps_sum <- -sum*mu; ps_sq <- sum2-sum*mu = D*var
        nmu_row = pools['ln'].tile([1, 512], BF16, tag="lnnmu")
        nc.vector.tensor_scalar_mul(nmu_row[:, :ncols], ps_sum[0:1, :ncols],
                                    -1.0 / D)
        # TensorTensor may read only one PSUM operand: stage -sum*mu in SBUF
        prod_row = pools['ln'].tile([1, 512], BF16, tag="lnprod")
        nc.vector.tensor_tensor(out=prod_row[:, :ncols],
                                in0=ps_sum[0:1, :ncols],
                                in1=nmu_row[:, :ncols], op=ALU.mult)
        nc.vector.tensor_tensor(out=ps_sq[0:1, :ncols],
                                in0=ps_sq[0:1, :ncols],
                                in1=prod_row[:, :ncols], op=ALU.add)
        # rstd = exp(-0.5*ln(var+eps)); Ln and Exp share an ACT table set.
        nc.scalar.activation(ps_sq[0:1, :ncols], ps_sq[0:1, :ncols], AF.Ln,
                             bias=self.eps_col[0:1], scale=1.0 / D)
        rstd_row = pools['ln'].tile([1, 512], BF16, tag="lnrstd")
        nc.scalar.activation(rstd_row[:, :ncols], ps_sq[0:1, :ncols],
                             AF.Exp, scale=-0.5)
        rstd_b = pools['ln'].tile([128, 512], BF16, tag="lnrstdb")
        nc.gpsimd.partition_broadcast(rstd_b[:, :ncols], rstd_row[:, :ncols])
        # -mu rides xT row (2,64); the k=2 nhat mult turns it into -mu*rstd
        nc.sync.dma_start(out=xT[64:65, 2, cols], in_=nmu_row[:1, :ncols])
        nhat = pools['nhat'].tile([128, 3, 512], BF16, tag="nhat")
        for k in range(3):
            rows = 128 if k < 2 else 65
            nc.vector.tensor_tensor(out=nhat[:rows, k, :ncols],
                                    in0=xT[:rows, k, cols],
                                    in1=rstd_b[:rows, :ncols], op=ALU.mult)
        if ff:
            nc.sync.dma_start(out=nhat[65:66, 2, :ncols],
                              in_=self.ones_row_bf[:1, :ncols])
        return nhat

    # ---------------- q/k/v ----------------
    def qkv_chunk(self, c0, w_q, w_k, w_v, qT, kT, vP, bias_plane):
        """LN + q/k/v for tokens [c0, c0+chunk): qT,kT [128,4,chunk] bf16
        c-major head-padded; vP [128, chunk/128, 328] bf16 token-major.
        bias_plane: index into qkbias for this stage's q bias (k = +1)."""
        nc = self.nc
        pools = self.pools
        qkb = self.w['qkbias']
        ntt = self.chunk // 512
        for n in range(ntt):
            tok0 = c0 + n * 512
            nhat = self.ln_nhat(tok0, 512)
            for wsb, dst, eng, bp in ((w_q, qT, 'act', bias_plane),
                                      (w_k, kT, 'dve', bias_plane + 1)):
                for mt in range(4):
                    ps = pools['psum'].tile([128, 512], F32, tag=f"p{'ab'[mt % 2]}")
                    for ki, (k, rows) in enumerate(self.KS_AUG):
                        nc.tensor.matmul(ps,
                                         wsb[:rows, k, mt * 128: mt * 128 + 128],
                                         nhat[:rows, k, :],
                                         start=(ki == 0), stop=(ki == 2))
                    dcols = slice(n * 512, (n + 1) * 512)
                    if eng == 'act':
                        # Identity (not Copy) so the per-partition beta bias
                        # is legal: out = in + bias
                        nc.scalar.activation(dst[:, mt, dcols], ps, AF.Identity,
                                             bias=qkb[:, bp, mt: mt + 1])
                    else:
                        nc.vector.tensor_scalar_add(dst[:, mt, dcols], ps,
                                                    qkb[:, bp, mt: mt + 1])
            for m in range(4):
                ps = pools['psum'].tile([128, 512], F32, tag=f"p{'ab'[m % 2]}")
                for ki, (k, rows) in enumerate(self.KS_AUG):
                    nc.tensor.matmul(ps[:, :328],
                                     nhat[:rows, k, m * 128:(m + 1) * 128],
                                     w_v[:rows, k, :328],
                                     start=(ki == 0), stop=(ki == 2))
                blk = n * 4 + m
                if m % 2 == 0:
                    nc.scalar.activation(vP[:, blk, :328], ps[:, :328], AF.Copy)
                else:
                    nc.vector.tensor_copy(out=vP[:, blk, :328], in_=ps[:, :328])
        # ones columns: vP[:, :, 32::41]
        onescols = mkap(vP, 32, [[vP.ap[0][0], 128],
                                 [328, self.chunk // 128], [41, 8]])
        nc.gpsimd.memset(onescols, 1.0)

    def finish_v2(self, chunkbuf, n512):
        """normalize chunkbuf[:, :4, cs]: per-head softmax denominators sit
        at rows 32 (lo) / 96 (hi) of each plane; gather with two DMAs into
        [8, 512], reciprocal, spread back with one K=8 selector matmul per
        plane, apply with one tensor_tensor per plane."""
        nc = self.nc
        pools = self.pools
        cs = slice(n512 * 512, (n512 + 1) * 512)
        rec8 = pools['attn'].tile([8, 512], BF16, tag="rec8")
        nc.sync.dma_start(out=rec8[0:4, :], in_=chunkbuf[32:33, 0:4, cs])
        nc.sync.dma_start(out=rec8[4:8, :], in_=chunkbuf[96:97, 0:4, cs])
        recf = pools['attn'].tile([8, 512], BF16, tag="recf")
        with nc.allow_low_precision(reason="softmax denom recip; 2e-2 tol"):
            nc.vector.reciprocal(recf, rec8)
        for pt in range(4):
            ps = pools['psum'].tile([128, 512], F32, tag=f"p{'ab'[pt % 2]}")
            nc.tensor.matmul(ps, self.selw[:, pt, :], recf,
                             start=True, stop=True)
            nc.vector.tensor_tensor(out=chunkbuf[:, pt, cs],
                                    in0=chunkbuf[:, pt, cs],
                                    in1=ps, op=ALU.mult)

    def wo_residual(self, chunkbuf, w_o, c0, n512):
        """xT[:, :, cols] += Wo_pad^T @ chunkbuf-slice (+bo via ptile 3 ones)."""
        nc = self.nc
        pools = self.pools
        cs = slice(n512 * 512, (n512 + 1) * 512)
        xcols = slice(c0 + n512 * 512, c0 + (n512 + 1) * 512)
        nc.sync.dma_start(out=chunkbuf[63:64, 3, cs],
                          in_=self.ones_row_bf[:1, :512])
        for mt in range(3):
            mrows = 128 if mt < 2 else 64
            ps = pools['psum'].tile([128, 512], F32, tag=f"p{'ab'[mt % 2]}")
            for k in range(4):
                nc.tensor.matmul(ps[:mrows, :],
                                 w_o[:, k, mt * 128: mt * 128 + mrows],
                                 chunkbuf[:, k, cs],
                                 start=(k == 0), stop=(k == 3))
            nc.vector.tensor_tensor(out=self.xT[:mrows, mt, xcols],
                                    in0=ps[:mrows, :],
                                    in1=self.xT[:mrows, mt, xcols], op=ALU.add)

    # ---------------- stage A: window attention ----------------
    def stage_window(self, c0):
        nc = self.nc
        pools = self.pools
        w_q, w_k = self.w['a1_wq'], self.w['a1_wk']
        w_v, w_o = self.w['a1_wv'], self.w['a1_wo']

        qT = pools['qkv'].tile([128, 4, self.chunk], BF16, tag="qT")
        kT = pools['qkv'].tile([128, 4, self.chunk], BF16, tag="kT")
        vP = pools['big1'].tile([128, self.chunk // 128, 328], BF16, tag="vP")
        self.qkv_chunk(c0, w_q, w_k, w_v, qT, kT, vP, 0)
        for wpair in range(self.chunk // 512):
            chunkbuf = pools['cb'].tile([128, 4, 512], BF16, tag="chunkbufA")
            nc.gpsimd.memset(chunkbuf, 0.0)
            for wi in range(2):
                t0 = wpair * 512 + wi * SEQ_TOK
                ccols = slice(wi * SEQ_TOK, (wi + 1) * SEQ_TOK)
                for pt in range(4):
                    sps = [pools['psum'].tile([128, 2, SEQ_TOK], F32,
                                              tag=f"p{'ef'[_hl]}",
                                              name=f"spA{_hl}")
                           for _hl in range(2)]
                    for mt in range(2):
                        for hl in range(2):
                            r0 = hl * 64
                            nc.tensor.matmul(
                                sps[hl][:, mt, :],
                                kT[r0:r0 + DH, pt,
                                   t0 + mt * 128: t0 + (mt + 1) * 128],
                                qT[r0:r0 + DH, pt, t0: t0 + SEQ_TOK],
                                start=True, stop=True)
                    eps = []
                    for hl in range(2):
                        ep = pools['ep'].tile([128, 2, SEQ_TOK], BF16, tag="ep")
                        nc.scalar.activation(ep, sps[hl], AF.Exp, scale=SCALE)
                        eps.append(ep)
                    avp = pools['psum'].tile([128, SEQ_TOK], F32, tag="pg")
                    for mt in range(2):
                        for hl in range(2):
                            h = 2 * pt + hl
                            nc.tensor.matmul(
                                avp[hl * 64: hl * 64 + VDIM, :],
                                vP[:, (t0 // 128) + mt,
                                   h * VDIM: (h + 1) * VDIM],
                                eps[hl][:, mt, :],
                                start=(mt == 0), stop=(mt == 1),
                                tile_position=(0, hl * 64))
                    for hl in range(2):
                        r0 = hl * 64
                        nc.scalar.activation(chunkbuf[r0:r0 + VDIM, pt, ccols],
                                             avp[r0:r0 + VDIM, :], AF.Copy)
            self.finish_v2(chunkbuf, 0)
            self.wo_residual(chunkbuf, w_o, c0 + wpair * 512, 0)

    # ---------------- stage B/D: temporal attention ----------------
    def stage_temporal(self, c0, st):
        nc = self.nc
        pools = self.pools
        w_q, w_k = self.w[f'{st}_wq'], self.w[f'{st}_wk']
        w_v, w_o = self.w[f'{st}_wv'], self.w[f'{st}_wo']
        tabq, tvrep = self.w[f'{st}_tabq'], self.w[f'{st}_tvrep']
        bias_plane = 2 if st == 't1' else 4

        nseq_c = self.chunk // T_LEN          # sequences per chunk
        ngrp_c = self.chunk // 128            # 8-seq groups per chunk
        qT = pools['qkv'].tile([128, 4, self.chunk], BF16, tag="qT")
        kT = pools['qkv'].tile([128, 4, self.chunk], BF16, tag="kT")
        vP = pools['big1'].tile([128, self.chunk // 128, 328], BF16, tag="vP")
        self.qkv_chunk(c0, w_q, w_k, w_v, qT, kT, vP, bias_plane)

        # qaug[plane h//4, (h%4)*32+J, i*nseq_c + seq]
        #   = q_h[:, tok(seq,i)] . tabQ[:, i*16+J]
        qaug = pools['big1'].tile([128, 2, T_LEN * nseq_c], BF16, tag="qaug")
        i_per = 512 // nseq_c
        for plane in range(2):
            for r in range(T_LEN // i_per):
                ps = pools['psum'].tile([128, 512], F32, tag=f"p{'ab'[r % 2]}")
                for ii in range(i_per):
                    i = r * i_per + ii
                    for hh in range(4):
                        h = plane * 4 + hh
                        pt, r0 = hrow(h)
                        nc.tensor.matmul(
                            ps[hh * 32: hh * 32 + 16,
                               ii * nseq_c:(ii + 1) * nseq_c],
                            tabq[r0:r0 + DH, i * 16:(i + 1) * 16],
                            qT[r0:r0 + DH, pt, i::T_LEN],
                            start=True, stop=True,
                            tile_position=(r0, hh * 32))
                for hh in range(4):
                    nc.scalar.activation(
                        qaug[hh * 32: hh * 32 + 16, plane,
                             r * 512:(r + 1) * 512],
                        ps[hh * 32: hh * 32 + 16, :], AF.Copy)

        chunkbuf = pools['big1'].tile([128, 4, self.chunk], BF16, tag="chunkbufT")
        nc.gpsimd.memset(chunkbuf, 0.0)
        for pt in range(4):
            eps = [pools['ep'].tile([128, self.chunk], BF16, tag="ep",
                                    name=f"ep{_hl}") for _hl in range(2)]
            for gq in range(ngrp_c // 4):
                sps = [pools['psum'].tile([128, 4, 128], F32,
                                          tag=f"p{'ef'[_hl]}",
                                          name=f"spT{_hl}")
                       for _hl in range(2)]
                for gg in range(4):
                    g = gq * 4 + gg
                    t0 = g * 128
                    for hl in range(2):
                        h = 2 * pt + hl
                        r0 = hl * 64
                        qb = (h % 4) * 32
                        plane = h // 4
                        nc.tensor.matmul(sps[hl][:, gg, :],
                                         kT[r0:r0 + DH, pt, t0:t0 + 128],
                                         qT[r0:r0 + DH, pt, t0:t0 + 128],
                                         start=True, stop=False)
                        rhs = mkap(qaug, qb * qaug.ap[0][0]
                                   + plane * qaug.ap[1][0] + g * 8,
                                   [[qaug.ap[0][0], 16], [1, 8], [nseq_c, 16]])
                        nc.tensor.matmul(sps[hl][:, gg, :],
                                         self.kaug[qb:qb + 16, :],
                                         rhs, start=False, stop=True,
                                         tile_position=(qb, 0))
                for hl in range(2):
                    nc.vector.tensor_tensor(out=sps[hl], in0=sps[hl],
                                            in1=self.mask, op=ALU.add)
                    epv = mkap(eps[hl], gq * 512,
                               [[eps[hl].ap[0][0], 128], [128, 4], [1, 128]])
                    nc.scalar.activation(epv, sps[hl], AF.Exp, scale=SCALE)
            for gq in range(ngrp_c // 4):
                avp = pools['psum'].tile([128, 4, 128], F32, tag="pg")
                for gg in range(4):
                    g = gq * 4 + gg
                    for hl in range(2):
                        h = 2 * pt + hl
                        nc.tensor.matmul(
                            avp[hl * 64: hl * 64 + VDIM, gg, :],
                            vP[:, g, h * VDIM: (h + 1) * VDIM],
                            eps[hl][:, g * 128:(g + 1) * 128],
                            start=True, stop=True,
                            tile_position=(0, hl * 64))
                for hl in range(2):
                    r0 = hl * 64
                    nc.scalar.activation(
                        chunkbuf[r0:r0 + VDIM, pt,
                                 gq * 512:(gq + 1) * 512],
                        avp[r0:r0 + VDIM, :, :], AF.Copy)
            # rel-v (writes 41 rows; the sum slot col of tvrep is zero),
            # added straight into chunkbuf (bf16) with strided APs.
            for hl in range(2):
                r0 = hl * 64
                for rr in range(T_LEN // i_per):
                    rvp = pools['psum'].tile([128, 512], F32, tag="ph")
                    for ii in range(i_per):
                        i = rr * i_per + ii
                        rb = (ii % 2) * 64
                        nc.tensor.matmul(
                            rvp[rb:rb + VDIM,
                                (ii // 2) * nseq_c:(ii // 2 + 1) * nseq_c],
                            tvrep[:, i * VDIM:(i + 1) * VDIM],
                            eps[hl][:, i::T_LEN], start=True, stop=True,
                            tile_position=(0, rb))
                    for par in range(2):
                        dst = mkap(chunkbuf, r0 * chunkbuf.ap[0][0]
                                   + pt * chunkbuf.ap[1][0]
                                   + rr * i_per + par,
                                   [[chunkbuf.ap[0][0], VDIM],
                                    [2, i_per // 2], [T_LEN, nseq_c]])
                        src_ = mkap(rvp, (par * 64) * rvp.ap[0][0],
                                    [[rvp.ap[0][0], VDIM],
                                     [nseq_c, i_per // 2], [1, nseq_c]])
                        nc.vector.tensor_tensor(out=dst, in0=dst, in1=src_,
                                                op=ALU.add)
        for n512 in range(self.chunk // 512):
            self.finish_v2(chunkbuf, n512)
            self.wo_residual(chunkbuf, w_o, c0, n512)

    # ---------------- stage C: cross attention ----------------
    def prep_cross_kv(self):
        nc = self.nc
        pools = self.pools
        w_k, w_v = self.w['a2_wk'], self.w['a2_wv']
        kT2 = pools['kv2'].tile([128, 4, NCTX], BF16, name="kT2")
        vP2 = pools['kv2'].tile([128, 328], BF16, name="vP2")
        for mt in range(4):
            ps = pools['psum'].tile([128, 512], F32, tag="pa")
            for k in range(6):
                nc.tensor.matmul(ps[:, :NCTX],
                                 w_k[:, k, mt * 128: mt * 128 + 128],
                                 self.ctxT[:, k, :],
                                 start=(k == 0), stop=(k == 5))
            nc.vector.tensor_copy(out=kT2[:, mt, :], in_=ps[:, :NCTX])
        ps = pools['psum'].tile([128, 512], F32, tag="pb")
        for k in range(6):
            nc.tensor.matmul(ps[:NCTX, :328], self.ctxT[:, k, :],
                             w_v[:, k, :328],
                             start=(k == 0), stop=(k == 5))
        nc.vector.tensor_copy(out=vP2[:NCTX, :], in_=ps[:NCTX, :328])
        onescols = mkap(vP2, 32, [[vP2.ap[0][0], NCTX], [VDIM, 8]])
        nc.gpsimd.memset(onescols, 1.0)
        self.kT2, self.vP2 = kT2, vP2

    def stage_cross(self, c0):
        nc = self.nc
        pools = self.pools
        w_q, w_o = self.w['a2_wq'], self.w['a2_wo']
        qkb = self.w['qkbias']
        kT2, vP2 = self.kT2, self.vP2

        qT = pools['qkv'].tile([128, 4, self.chunk], BF16, tag="qT")
        for n in range(self.chunk // 512):
            tok0 = c0 + n * 512
            nhat = self.ln_nhat(tok0, 512)
            for mt in range(4):
                ps = pools['psum'].tile([128, 512], F32, tag=f"p{'ab'[mt % 2]}")
                for ki, (k, rows) in enumerate(self.KS_AUG):
                    nc.tensor.matmul(ps,
                                     w_q[:rows, k, mt * 128: mt * 128 + 128],
                                     nhat[:rows, k, :],
                                     start=(ki == 0), stop=(ki == 2))
                nc.scalar.activation(qT[:, mt, n * 512:(n + 1) * 512],
                                     ps, AF.Identity, bias=qkb[:, 6, mt: mt + 1])
        for n in range(self.chunk // 512):
            ns = slice(n * 512, (n + 1) * 512)
            chunkbuf = pools['cb'].tile([128, 4, 512], BF16, tag="chunkbufA")
            nc.gpsimd.memset(chunkbuf, 0.0)
            for pt in range(4):
                sps = [pools['psum'].tile([128, 512], F32, tag=f"p{'ef'[_hl]}",
                                          name=f"spC{_hl}")
                       for _hl in range(2)]
                for hl in range(2):
                    r0 = hl * 64
                    nc.tensor.matmul(sps[hl][:NCTX, :],
                                     kT2[r0:r0 + DH, pt, :],
                                     qT[r0:r0 + DH, pt, ns],
                                     start=True, stop=True)
                eps = []
                for hl in range(2):
                    ep = pools['ep'].tile([128, 512], BF16, tag="ep")
                    nc.scalar.activation(ep[:NCTX, :], sps[hl][:NCTX, :],
                                         AF.Exp, scale=SCALE)
                    eps.append(ep)
                avp = pools['psum'].tile([128, 512], F32, tag="pg")
                for hl in range(2):
                    h = 2 * pt + hl
                    nc.tensor.matmul(avp[hl * 64: hl * 64 + VDIM, :],
                                     vP2[:NCTX, h * VDIM: (h + 1) * VDIM],
                                     eps[hl][:NCTX, :], start=True, stop=True,
                                     tile_position=(0, hl * 64))
                for hl in range(2):
                    r0 = hl * 64
                    nc.scalar.activation(chunkbuf[r0:r0 + VDIM, pt, :],
                                         avp[r0:r0 + VDIM, :], AF.Copy)
            self.finish_v2(chunkbuf, 0)
            self.wo_residual(chunkbuf, w_o, c0 + n * 512, 0)

    # ---------------- stage E: GEGLU FF ----------------
    def stage_ff(self, c0):
        nc = self.nc
        pools = self.pools
        w1, w2 = self.w['ff_w1'], self.w['ff_w2']

        for n in range(self.chunk // 512):
            tok0 = c0 + n * 512
            nhat = self.ln_nhat(tok0, 512, ff=True)
            # w2 accumulation runs inline over the 10 gelu planes; psum
            # banks pe/pf/pg hold the three 128-row output strips.
            w2ps = [pools['psum'].tile([128, 512], F32, tag=f"p{'efg'[mt]}",
                                       name=f"w2ps{mt}")
                    for mt in range(3)]
            for mt in range(10):
                aps = pools['psum'].tile([128, 512], F32, tag="pa")
                gps = pools['psum'].tile([128, 512], F32, tag="pb")
                for ki, (k, rows) in enumerate(self.KS_FF):
                    nc.tensor.matmul(aps,
                                     w1[:rows, k, mt * 128: mt * 128 + 128],
                                     nhat[:rows, k, :],
                                     start=(ki == 0), stop=(ki == 2))
                    nc.tensor.matmul(gps,
                                     w1[:rows, k, FF + mt * 128: FF + mt * 128 + 128],
                                     nhat[:rows, k, :],
                                     start=(ki == 0), stop=(ki == 2))
                gelu = pools['ln'].tile([128, 512], BF16, tag="gelu")
                nc.scalar.activation(gelu, gps, AF.Gelu)
                ffk = pools['ff'].tile([128, 512], BF16, tag="ffk")
                nc.vector.tensor_tensor(out=ffk, in0=aps, in1=gelu,
                                        op=ALU.mult)
                for m2 in range(3):
                    m2rows = 128 if m2 < 2 else 64
                    nc.tensor.matmul(w2ps[m2][:m2rows, :],
                                     w2[:, mt, m2 * 128: m2 * 128 + m2rows],
                                     ffk, start=(mt == 0), stop=False)
            for m2 in range(3):
                m2rows = 128 if m2 < 2 else 64
                nc.tensor.matmul(w2ps[m2][:m2rows, :],
                                 w2[0:1, 10, m2 * 128: m2 * 128 + m2rows],
                                 self.ones_row_bf[0:1, :],
                                 start=False, stop=True)
                cols = slice(tok0, tok0 + 512)
                nc.vector.tensor_tensor(out=self.xT[:m2rows, m2, cols],
                                        in0=w2ps[m2][:m2rows, :],
                                        in1=self.xT[:m2rows, m2, cols],
                                        op=ALU.add)


# ----------------------------------------------------------------------------
# host entry point
# ----------------------------------------------------------------------------

_nc_cache = {}


def _get_nc(nwin=16, chunk_win=4):
    key = (nwin, chunk_win)
    if key not in _nc_cache:
        _nc_cache[key] = Builder(nwin, chunk_win).build(num_devices=NCORES)
    return _nc_cache[key]


def make_in_maps(inputs, nwin=16):
    x = np.asarray(inputs['x'], np.float32)
    context = np.asarray(inputs['context'], np.float32)
    wd = {k: np.asarray(v, np.float32) for k, v in inputs.items()
          if k not in ('x', 'context')}
    wt = prep_weights(wd)
    shards = shard_x(x, nwin)
    ncore = shards.shape[0]
    in_maps = []
    for c in range(ncore):
        bidx = (c * nwin) // (NH * NH)
        ctxT = _cmajor(np.ascontiguousarray(context[bidx].T), 768)  # [128,6,77]
        m = {'xT': np.ascontiguousarray(_cmajor(shards[c], 384)).astype(bfdt),
             'ctxT': np.ascontiguousarray(ctxT).astype(bfdt)}
        m.update(wt)
        in_maps.append(m)
    return in_maps


def kernel(**inputs):
    nwin = 16
    nc = _get_nc(nwin)
    in_maps = make_in_maps(inputs, nwin)
    res = run_bass_kernel_spmd(nc, in_maps, list(range(NCORES)))
    outs = np.stack([np.asarray(r['yT'], np.float32) for r in res.results])
    # undo c-major padding: [8, 128, 3, ntok] -> [8, 384, ntok] -> [8, 320, ntok]
    outs = outs.transpose(0, 2, 1, 3).reshape(NCORES, 384, nwin * SEQ_TOK)[:, :D]
    return unshard_x(outs, nwin).astype(np.float32)
